# revision 33
# baseline (speedup 1.0000x reference)
"""Trainium2 Bass kernel for the Decoder (gather + shared-MLP over agents).

Math:
  assigned[b,n] = abs_actions[b, assign[b,n]]                    (gather, A=16)
  out[b,n,:]    = relu(assigned[b,n]*W1[0,:] + emb[n,:]@W1[1:,:] + b1) @ W2 + b2

Factorization (N sharded 8 ways -> NC=1250 per core, h on partitions):
  embW[n,h]   = emb[n,:]@W1[1:,h] (+ b1 folded into emb on host when nonzero)
  relu(x + e) = max(x, -e) + e
  out[b,n,o]  = sum_h max(s[b,n]*w0[h], -embW[n,h]) * W2[h,o] + corr[n,o]
              = sum_h max(sign(w0[h])*s[b,n], q[n,h]) * W2'[h,o] + corr[n,o]
    with q = -embW/|w0|, W2' = |w0|*W2, corr = embW@W2 + b2 (batch-indep).

The sign form lets the per-partition scale ride for free: H is permuted so
each of the 128 partitions holds two h's of EQUAL sign(w0) (one per K-half),
partitions sorted [positive-sign block | negative-sign block].  The device
then needs only (a) broadcast s to the positive block and -s to the negative
block (GPSIMD, bitcast to f32 to halve element count), and (b) ONE fused
tensor_tensor max per batch (DVE 2x bf16).  h's that break the pairing
(odd sign count, |w0| ~ 0) are excluded from the device and their exact
contribution plus corr is added on the host.

Device pipeline per batch b:
  DMA    : s_all[b]/(-s) row pair -> partition 0 staging (one small DMA)
  GPSIMD : two partition_broadcasts (f32-viewed) fill sbc[128, NP]
  DVE    : u = max(sbc, q) over both K-halves in one op (2x bf16)
  PE     : pso[32j+o, nn] = sum_h u[h, 320j+nn]*W2'[h,o] (8 bf16 matmuls,
           col-packed 4x via tile_position)
  ACT    : drain PSUM -> SBUF (f32); one [128,320] DMA out (row model:
           cost ~ bytes per partition line, so full-width is cheap)

The gather is a one-hot matmul: onehot[(b,a), n] = (assign[b,n]==a) built by
one DVE compare per 128-row tile, contracted with a block-diagonal
abs_actions matrix on the PE.
"""

import sys

sys.path.insert(0, "/opt/trn_rl_repo")

import numpy as np
import ml_dtypes

import concourse.bass as bass
import concourse.tile as tile
import concourse.mybir as mybir
from concourse import bacc
from concourse.bass_utils import run_bass_kernel_spmd

BF16 = ml_dtypes.bfloat16

B, A, N, E, H, OUT = 32, 16, 10000, 256, 256, 2
NCORES = 8
NC = N // NCORES  # 1250 real columns per core
NP = 1280  # padded to 4 * 320 for regular chunking
P = 128

CH = [0, 512, 1024, NP]  # chunks for gather matmuls staged through PSUM
CG = [0, 320, 640, 960, NP]  # column groups for the col-packed consume

_CACHE = {}


def build_program():
    """Build the Bass/Tile program once (shared by all 8 cores, SPMD)."""
    nc = bacc.Bacc("TRN2", target_bir_lowering=False, debug=False)
    f32 = mybir.dt.float32
    bf16 = mybir.dt.bfloat16

    d_q = nc.dram_tensor("qtab", (2, P, NP), bf16, kind="ExternalInput").ap()
    d_arep = nc.dram_tensor("assign_rep", (4, P, NP), bf16, kind="ExternalInput").ap()
    d_absf = nc.dram_tensor("absflat", (4, P, B), bf16, kind="ExternalInput").ap()
    d_iota = nc.dram_tensor("iota16", (P, 1), f32, kind="ExternalInput").ap()
    d_sig = nc.dram_tensor("sigma", (P, 1), f32, kind="ExternalInput").ap()
    d_w2p = nc.dram_tensor("w2p", (2, P, 32), bf16, kind="ExternalInput").ap()
    d_out = nc.dram_tensor("out", (B, P, 320), bf16, kind="ExternalOutput").ap()

    mm = mybir.AluOpType

    with tile.TileContext(nc) as tc:
        with (
            tc.tile_pool(name="const", bufs=1) as cpool,
            tc.tile_pool(name="work", bufs=1) as wpool,
            tc.tile_pool(name="sfl", bufs=4) as sflp,
            tc.tile_pool(name="sbc", bufs=4) as sbcp,
            tc.tile_pool(name="sbs", bufs=4) as sbsp,
            tc.tile_pool(name="tt", bufs=4) as ttp,
            tc.tile_pool(name="ostg", bufs=4) as ostgp,
            tc.tile_pool(name="ps_pro", bufs=2, space="PSUM") as pspro,
            tc.tile_pool(name="ps_out", bufs=4, space="PSUM") as psout,
        ):
            # ---- load constants / inputs ----
            qtab = cpool.tile([P, 2, NP], bf16)
            arep = cpool.tile([P, 4, NP], bf16)
            absf = cpool.tile([P, 4, B], bf16)
            iota = cpool.tile([P, 1], f32)
            sig = cpool.tile([P, 1], f32)
            w2p = cpool.tile([P, 2, 32], bf16)

            for k in range(2):
                nc.sync.dma_start(qtab[:, k, :], d_q[k])
                nc.sync.dma_start(w2p[:, k, :], d_w2p[k])
            for t in range(4):
                nc.sync.dma_start(arep[:, t, :], d_arep[t])
                nc.scalar.dma_start(absf[:, t, :], d_absf[t])
            nc.scalar.dma_start(iota[:], d_iota[:])
            nc.scalar.dma_start(sig[:], d_sig[:])

            # ---- working tensors ----
            onehot = wpool.tile([P, 4, NP], bf16)
            s_all = wpool.tile([B, NP], bf16)

            # ---- one-hot of assignments: onehot[(b,a), n] = (assign[b,n]==a)
            for t in range(4):
                nc.vector.tensor_scalar(
                    onehot[:, t, :], arep[:, t, :], iota[:, 0:1], None, mm.is_equal
                )

            # ---- gather s[b,n] = abs_actions[b, assign[b,n]] via matmul ----
            for ci in range(3):
                w = CH[ci + 1] - CH[ci]
                ps = pspro.tile([B, 512], f32, tag="ps_s")
                for t in range(4):
                    nc.tensor.matmul(
                        ps[:, :w],
                        absf[:, t, :],
                        onehot[:, t, CH[ci] : CH[ci + 1]],
                        start=(t == 0),
                        stop=(t == 3),
                    )
                nc.scalar.copy(s_all[:, CH[ci] : CH[ci + 1]], ps[:, :w])

            # ---- main loop over batches ----
            for b in range(B):
                # stage s[b,:] at partition 0 (tiny DMA), broadcast on GPSIMD
                sfl = sflp.tile([1, NP], bf16, tag="sfl")
                nc.scalar.dma_start(sfl[:], s_all[b : b + 1, :])
                sbc = sbcp.tile([P, NP], bf16, tag="sbc")
                nc.gpsimd.partition_broadcast(sbc[:], sfl[0:1, :])

                # u = max(sign*s, q): one fused scalar_tensor_tensor over
                # both K-halves — (sbc * sigma) max q
                tt = ttp.tile([P, 2, NP], bf16, tag="tt")
                nc.vector.scalar_tensor_tensor(
                    tt[:],
                    sbc[:].unsqueeze(1).broadcast_to([P, 2, NP]),
                    sig[:, 0:1],
                    qtab[:],
                    mm.mult,
                    mm.max,
                )

                pso = psout.tile([P, 320], f32, tag="pso")
                for j in range(4):
                    for k in range(2):
                        nc.tensor.matmul(
                            pso[32 * j : 32 * j + 32, :],
                            w2p[:, k, :],
                            tt[:, k, CG[j] : CG[j + 1]],
                            start=(k == 0),
                            stop=(k == 1),
                            tile_position=(0, 32 * j),
                        )

                ostg = ostgp.tile([P, 320], bf16, tag="ostg")
                nc.scalar.copy(ostg[:], pso[:])
                nc.sync.dma_start(d_out[b], ostg[:])

    nc.compile()
    return nc


def prep_inputs(abs_actions, assignments, q_host):
    """Per-core input dicts. q_host: (2, 128, N) bf16 permuted q table."""
    in_maps = []
    for c in range(NCORES):
        sl = slice(c * NC, (c + 1) * NC)
        a_sl = np.zeros((B, NP), np.int32)
        a_sl[:, :NC] = assignments[:, sl]
        arep = np.ascontiguousarray(
            a_sl[np.arange(B * A) // A].reshape(4, P, NP)
        ).astype(BF16)
        qc = np.zeros((2, P, NP), BF16)
        qc[:, :, :NC] = q_host[:, :, sl]
        in_maps.append(
            {
                "qtab": qc,
                "assign_rep": arep,
                "absflat": _CACHE["absflat"],
                "iota16": _CACHE["iota16"],
                "sigma": _CACHE["sigma"],
                "w2p": _CACHE["w2p"],
            }
        )
    return in_maps


def kernel(abs_actions, abstract_agent_assignments, emb, W1, b1, W2, b2):
    abs_actions = np.asarray(abs_actions, np.float32)
    assign = np.asarray(abstract_agent_assignments).astype(np.int32)
    emb = np.asarray(emb, np.float32)
    W1 = np.asarray(W1, np.float32)
    b1 = np.asarray(b1, np.float32)
    W2 = np.asarray(W2, np.float32)
    b2 = np.asarray(b2, np.float32)

    # Fold b1 into emb: (emb + 1 v^T) @ W1[1:] = emb@W1[1:] + 1 b1^T when
    # W1[1:].T v = b1.  Exact for full-rank square W1[1:]; b1 == 0 here anyway.
    if np.any(b1 != 0):
        v = np.linalg.lstsq(W1[1:].T, b1, rcond=None)[0]
        if not np.allclose(W1[1:].T @ v, b1, atol=1e-5):
            raise ValueError("cannot fold nonzero b1 exactly")
        emb = emb + v[None, :]

    # Weight-only precomputes.
    w0 = W1[0, :]  # (H,)
    embW = emb @ W1[1:]  # (N, H) f32
    corr = embW @ W2 + b2  # (N, OUT) f32

    # Partition h's by sign(w0); exclude degenerate/odd-parity h's to host.
    absw0 = np.abs(w0)
    tiny = absw0 < 1e-5 * max(absw0.max(), 1e-30)
    pos = [h for h in range(H) if w0[h] > 0 and not tiny[h]]
    neg = [h for h in range(H) if not (w0[h] > 0) and not tiny[h]]
    host_hs = list(np.nonzero(tiny)[0])
    if len(pos) % 2 == 1:
        host_hs.append(pos.pop())
    if len(neg) % 2 == 1:
        host_hs.append(neg.pop())
    # pairs: partition p gets two h's of equal sign(w0); sigma[p] carries the
    # common sign, dummy (-1) pairs pad to 128 partitions
    npos, nneg = len(pos) // 2, len(neg) // 2
    pairs = (
        [(pos[2 * i], pos[2 * i + 1]) for i in range(npos)]
        + [(neg[2 * i], neg[2 * i + 1]) for i in range(nneg)]
    )
    pairs += [(-1, -1)] * (P - len(pairs))
    sigma = np.ones((P, 1), np.float32)
    sigma[npos : npos + nneg] = -1.0
    assert len(pairs) == P
    perm = np.zeros((2, P), np.int64)  # perm[k][p] = h index (or -1 dummy)
    valid = np.zeros((2, P), bool)
    for p, (h0, h1) in enumerate(pairs):
        if h0 >= 0:
            perm[0][p], perm[1][p] = h0, h1
            valid[0][p] = valid[1][p] = True

    # q[k][p][n] = -embW[n, perm[k][p]] / |w0[perm[k][p]]| ; dummies get 0
    q_host = np.zeros((2, P, N), np.float32)
    w2p = np.zeros((2, P, 32), np.float32)
    for k in range(2):
        hs = perm[k][valid[k]]
        q_host[k][valid[k]] = (-embW[:, hs] / absw0[hs][None, :]).T
        w2p[k][valid[k], :OUT] = absw0[hs][:, None] * W2[hs, :]
    q_host = q_host.astype(BF16)

    _build_consts(abs_actions, w2p)
    _CACHE["sigma"] = sigma

    if "nc" not in _CACHE:
        _CACHE["nc"] = build_program()
    nc = _CACHE["nc"]

    in_maps = prep_inputs(abs_actions, assign, q_host)
    _CACHE["in_maps"] = in_maps
    res = run_bass_kernel_spmd(nc, in_maps, list(range(NCORES))).results
    outs = np.stack(
        [np.asarray(res[c]["out"]).astype(np.float32) for c in range(NCORES)]
    )
    # outs: (8, B, 128, 320); row 32j+o, col nn -> out[b, c*1250 + 320j + nn, o]
    outs = outs.reshape(NCORES, B, 4, 32, 320)[:, :, :, :OUT, :]  # (8,B,4,2,320)
    outs = outs.transpose(1, 0, 2, 4, 3).reshape(B, NCORES, NP, OUT)
    out = np.ascontiguousarray(outs[:, :, :NC, :].reshape(B, N, OUT))

    # host-side: corr + any h's excluded from the device pairing (exact f32)
    out += corr[None]
    if host_hs:
        s = np.take_along_axis(abs_actions, assign, axis=1)  # (B, N)
        for h in host_hs:
            relu_h = np.maximum(s * w0[h], -embW[:, h][None, :])  # (B, N)
            out += relu_h[:, :, None] * W2[h][None, None, :]
    return out


def _build_consts(abs_actions, w2p):
    absflat = np.zeros((B * A, B), np.float32)
    absflat[np.arange(B * A), np.arange(B * A) // A] = abs_actions.reshape(-1)
    _CACHE["absflat"] = np.ascontiguousarray(absflat.reshape(4, P, B)).astype(BF16)
    _CACHE["iota16"] = (np.arange(P, dtype=np.float32) % A).reshape(P, 1)
    _CACHE["w2p"] = np.ascontiguousarray(w2p).astype(BF16)


# revision 35
# speedup vs baseline: 1.0073x; 1.0073x over previous
"""Trainium2 Bass kernel for the Decoder (gather + shared-MLP over agents).

Math:
  assigned[b,n] = abs_actions[b, assign[b,n]]                    (gather, A=16)
  out[b,n,:]    = relu(assigned[b,n]*W1[0,:] + emb[n,:]@W1[1:,:] + b1) @ W2 + b2

Factorization (N sharded 8 ways -> NC=1250 per core, h on partitions):
  embW[n,h]   = emb[n,:]@W1[1:,h] (+ b1 folded into emb on host when nonzero)
  relu(x + e) = max(x, -e) + e
  out[b,n,o]  = sum_h max(s[b,n]*w0[h], -embW[n,h]) * W2[h,o] + corr[n,o]
              = sum_h max(sign(w0[h])*s[b,n], q[n,h]) * W2'[h,o] + corr[n,o]
    with q = -embW/|w0|, W2' = |w0|*W2, corr = embW@W2 + b2 (batch-indep).

The sign form lets the per-partition scale ride for free: H is permuted so
each of the 128 partitions holds two h's of EQUAL sign(w0) (one per K-half),
partitions sorted [positive-sign block | negative-sign block].  The device
then needs only (a) broadcast s to the positive block and -s to the negative
block (GPSIMD, bitcast to f32 to halve element count), and (b) ONE fused
tensor_tensor max per batch (DVE 2x bf16).  h's that break the pairing
(odd sign count, |w0| ~ 0) are excluded from the device and their exact
contribution plus corr is added on the host.

Device pipeline per batch b:
  DMA    : s_all[b]/(-s) row pair -> partition 0 staging (one small DMA)
  GPSIMD : two partition_broadcasts (f32-viewed) fill sbc[128, NP]
  DVE    : u = max(sbc, q) over both K-halves in one op (2x bf16)
  PE     : pso[32j+o, nn] = sum_h u[h, 320j+nn]*W2'[h,o] (8 bf16 matmuls,
           col-packed 4x via tile_position)
  ACT    : drain PSUM -> SBUF (f32); one [128,320] DMA out (row model:
           cost ~ bytes per partition line, so full-width is cheap)

The gather is a one-hot matmul: onehot[(b,a), n] = (assign[b,n]==a) built by
one DVE compare per 128-row tile, contracted with a block-diagonal
abs_actions matrix on the PE.
"""

import sys

sys.path.insert(0, "/opt/trn_rl_repo")

import numpy as np
import ml_dtypes

import concourse.bass as bass
import concourse.tile as tile
import concourse.mybir as mybir
from concourse import bacc
from concourse.bass_utils import run_bass_kernel_spmd

BF16 = ml_dtypes.bfloat16

B, A, N, E, H, OUT = 32, 16, 10000, 256, 256, 2
NCORES = 8
NC = N // NCORES  # 1250 real columns per core
NP = 1280  # padded to 4 * 320 for regular chunking
P = 128

CH = [0, 512, 1024, NP]  # chunks for gather matmuls staged through PSUM
CG = [0, 320, 640, 960, NP]  # column groups for the col-packed consume

_CACHE = {}


def build_program():
    """Build the Bass/Tile program once (shared by all 8 cores, SPMD)."""
    nc = bacc.Bacc("TRN2", target_bir_lowering=False, debug=False)
    f32 = mybir.dt.float32
    bf16 = mybir.dt.bfloat16

    d_q = nc.dram_tensor("qtab", (2, P, NP), bf16, kind="ExternalInput").ap()
    d_arep = nc.dram_tensor("assign_rep", (4, P, NP), bf16, kind="ExternalInput").ap()
    d_absf = nc.dram_tensor("absflat", (4, P, B), bf16, kind="ExternalInput").ap()
    d_iota = nc.dram_tensor("iota16", (P, 1), f32, kind="ExternalInput").ap()
    d_sig = nc.dram_tensor("sigma", (P, 1), f32, kind="ExternalInput").ap()
    d_w2p = nc.dram_tensor("w2p", (2, P, 32), bf16, kind="ExternalInput").ap()
    d_out = nc.dram_tensor("out", (B, P, 320), bf16, kind="ExternalOutput").ap()

    mm = mybir.AluOpType

    with tile.TileContext(nc) as tc:
        with (
            tc.tile_pool(name="const", bufs=1) as cpool,
            tc.tile_pool(name="work", bufs=1) as wpool,
            tc.tile_pool(name="sfl", bufs=4) as sflp,
            tc.tile_pool(name="sbc", bufs=4) as sbcp,
            tc.tile_pool(name="sbs", bufs=4) as sbsp,
            tc.tile_pool(name="tt", bufs=4) as ttp,
            tc.tile_pool(name="ostg", bufs=4) as ostgp,
            tc.tile_pool(name="ps_pro", bufs=2, space="PSUM") as pspro,
            tc.tile_pool(name="ps_out", bufs=4, space="PSUM") as psout,
        ):
            # ---- load constants / inputs ----
            qtab = cpool.tile([P, 2, NP], bf16)
            arep = cpool.tile([P, 4, NP], bf16)
            absf = cpool.tile([P, 4, B], bf16)
            iota = cpool.tile([P, 1], f32)
            sig = cpool.tile([P, 1], f32)
            w2p = cpool.tile([P, 2, 32], bf16)

            for k in range(2):
                nc.sync.dma_start(qtab[:, k, :], d_q[k])
                nc.sync.dma_start(w2p[:, k, :], d_w2p[k])
            for t in range(4):
                nc.sync.dma_start(arep[:, t, :], d_arep[t])
                nc.scalar.dma_start(absf[:, t, :], d_absf[t])
            nc.scalar.dma_start(iota[:], d_iota[:])
            nc.scalar.dma_start(sig[:], d_sig[:])

            # ---- working tensors ----
            onehot = wpool.tile([P, 4, NP], bf16)
            s_all = wpool.tile([B, NP], bf16)

            # ---- one-hot of assignments: onehot[(b,a), n] = (assign[b,n]==a)
            for t in range(4):
                nc.vector.tensor_scalar(
                    onehot[:, t, :], arep[:, t, :], iota[:, 0:1], None, mm.is_equal
                )

            # ---- gather s[b,n] = abs_actions[b, assign[b,n]] via matmul ----
            for ci in range(3):
                w = CH[ci + 1] - CH[ci]
                ps = pspro.tile([B, 512], f32, tag="ps_s")
                for t in range(4):
                    nc.tensor.matmul(
                        ps[:, :w],
                        absf[:, t, :],
                        onehot[:, t, CH[ci] : CH[ci + 1]],
                        start=(t == 0),
                        stop=(t == 3),
                    )
                nc.scalar.copy(s_all[:, CH[ci] : CH[ci + 1]], ps[:, :w])

            # ---- main loop over batches ----
            for b in range(B):
                # broadcast s[b,:] to 128 partitions: GPSIMD for 3 of every 4
                # batches (via a tiny partition-0 staging DMA), DMA fabric for
                # the rest (relieves the shared GPSIMD/DVE SBUF ports)
                sbc = sbcp.tile([P, NP], bf16, tag="sbc")
                if b % 4 != 3:
                    sfl = sflp.tile([1, NP], bf16, tag="sfl")
                    nc.scalar.dma_start(sfl[:], s_all[b : b + 1, :])
                    nc.gpsimd.partition_broadcast(sbc[:], sfl[0:1, :])
                else:
                    src = (
                        s_all[b : b + 1, :].unsqueeze(1).broadcast_to([1, P, NP])
                    )
                    nc.sync.dma_start(sbc[:], src)

                # apply per-partition sign; alternate DVE/ACT to balance load
                sbs = sbsp.tile([P, NP], bf16, tag="sbs")
                if b % 2 == 0:
                    nc.vector.tensor_scalar(
                        sbs[:], sbc[:], sig[:, 0:1], None, mm.mult
                    )
                else:
                    nc.scalar.activation(
                        sbs[:],
                        sbc[:],
                        mybir.ActivationFunctionType.Identity,
                        scale=sig[:, 0:1],
                    )

                # u = max(sign*s, q), both K-halves in one fused op
                tt = ttp.tile([P, 2, NP], bf16, tag="tt")
                nc.vector.tensor_tensor(
                    tt[:],
                    sbs[:].unsqueeze(1).broadcast_to([P, 2, NP]),
                    qtab[:],
                    mm.max,
                )

                pso = psout.tile([P, 320], f32, tag="pso")
                for j in range(4):
                    for k in range(2):
                        nc.tensor.matmul(
                            pso[32 * j : 32 * j + 32, :],
                            w2p[:, k, :],
                            tt[:, k, CG[j] : CG[j + 1]],
                            start=(k == 0),
                            stop=(k == 1),
                            tile_position=(0, 32 * j),
                        )

                ostg = ostgp.tile([P, 320], bf16, tag="ostg")
                nc.scalar.copy(ostg[:], pso[:])
                nc.sync.dma_start(d_out[b], ostg[:])

    nc.compile()
    return nc


def prep_inputs(abs_actions, assignments, q_host):
    """Per-core input dicts. q_host: (2, 128, N) bf16 permuted q table."""
    in_maps = []
    for c in range(NCORES):
        sl = slice(c * NC, (c + 1) * NC)
        a_sl = np.zeros((B, NP), np.int32)
        a_sl[:, :NC] = assignments[:, sl]
        arep = np.ascontiguousarray(
            a_sl[np.arange(B * A) // A].reshape(4, P, NP)
        ).astype(BF16)
        qc = np.zeros((2, P, NP), BF16)
        qc[:, :, :NC] = q_host[:, :, sl]
        in_maps.append(
            {
                "qtab": qc,
                "assign_rep": arep,
                "absflat": _CACHE["absflat"],
                "iota16": _CACHE["iota16"],
                "sigma": _CACHE["sigma"],
                "w2p": _CACHE["w2p"],
            }
        )
    return in_maps


def kernel(abs_actions, abstract_agent_assignments, emb, W1, b1, W2, b2):
    abs_actions = np.asarray(abs_actions, np.float32)
    assign = np.asarray(abstract_agent_assignments).astype(np.int32)
    emb = np.asarray(emb, np.float32)
    W1 = np.asarray(W1, np.float32)
    b1 = np.asarray(b1, np.float32)
    W2 = np.asarray(W2, np.float32)
    b2 = np.asarray(b2, np.float32)

    # Fold b1 into emb: (emb + 1 v^T) @ W1[1:] = emb@W1[1:] + 1 b1^T when
    # W1[1:].T v = b1.  Exact for full-rank square W1[1:]; b1 == 0 here anyway.
    if np.any(b1 != 0):
        v = np.linalg.lstsq(W1[1:].T, b1, rcond=None)[0]
        if not np.allclose(W1[1:].T @ v, b1, atol=1e-5):
            raise ValueError("cannot fold nonzero b1 exactly")
        emb = emb + v[None, :]

    # Weight-only precomputes.
    w0 = W1[0, :]  # (H,)
    embW = emb @ W1[1:]  # (N, H) f32
    corr = embW @ W2 + b2  # (N, OUT) f32

    # Partition h's by sign(w0); exclude degenerate/odd-parity h's to host.
    absw0 = np.abs(w0)
    tiny = absw0 < 1e-5 * max(absw0.max(), 1e-30)
    pos = [h for h in range(H) if w0[h] > 0 and not tiny[h]]
    neg = [h for h in range(H) if not (w0[h] > 0) and not tiny[h]]
    host_hs = list(np.nonzero(tiny)[0])
    if len(pos) % 2 == 1:
        host_hs.append(pos.pop())
    if len(neg) % 2 == 1:
        host_hs.append(neg.pop())
    # pairs: partition p gets two h's of equal sign(w0); sigma[p] carries the
    # common sign, dummy (-1) pairs pad to 128 partitions
    npos, nneg = len(pos) // 2, len(neg) // 2
    pairs = (
        [(pos[2 * i], pos[2 * i + 1]) for i in range(npos)]
        + [(neg[2 * i], neg[2 * i + 1]) for i in range(nneg)]
    )
    pairs += [(-1, -1)] * (P - len(pairs))
    sigma = np.ones((P, 1), np.float32)
    sigma[npos : npos + nneg] = -1.0
    assert len(pairs) == P
    perm = np.zeros((2, P), np.int64)  # perm[k][p] = h index (or -1 dummy)
    valid = np.zeros((2, P), bool)
    for p, (h0, h1) in enumerate(pairs):
        if h0 >= 0:
            perm[0][p], perm[1][p] = h0, h1
            valid[0][p] = valid[1][p] = True

    # q[k][p][n] = -embW[n, perm[k][p]] / |w0[perm[k][p]]| ; dummies get 0
    q_host = np.zeros((2, P, N), np.float32)
    w2p = np.zeros((2, P, 32), np.float32)
    for k in range(2):
        hs = perm[k][valid[k]]
        q_host[k][valid[k]] = (-embW[:, hs] / absw0[hs][None, :]).T
        w2p[k][valid[k], :OUT] = absw0[hs][:, None] * W2[hs, :]
    q_host = q_host.astype(BF16)

    _build_consts(abs_actions, w2p)
    _CACHE["sigma"] = sigma

    if "nc" not in _CACHE:
        _CACHE["nc"] = build_program()
    nc = _CACHE["nc"]

    in_maps = prep_inputs(abs_actions, assign, q_host)
    _CACHE["in_maps"] = in_maps
    res = run_bass_kernel_spmd(nc, in_maps, list(range(NCORES))).results
    outs = np.stack(
        [np.asarray(res[c]["out"]).astype(np.float32) for c in range(NCORES)]
    )
    # outs: (8, B, 128, 320); row 32j+o, col nn -> out[b, c*1250 + 320j + nn, o]
    outs = outs.reshape(NCORES, B, 4, 32, 320)[:, :, :, :OUT, :]  # (8,B,4,2,320)
    outs = outs.transpose(1, 0, 2, 4, 3).reshape(B, NCORES, NP, OUT)
    out = np.ascontiguousarray(outs[:, :, :NC, :].reshape(B, N, OUT))

    # host-side: corr + any h's excluded from the device pairing (exact f32)
    out += corr[None]
    if host_hs:
        s = np.take_along_axis(abs_actions, assign, axis=1)  # (B, N)
        for h in host_hs:
            relu_h = np.maximum(s * w0[h], -embW[:, h][None, :])  # (B, N)
            out += relu_h[:, :, None] * W2[h][None, None, :]
    return out


def _build_consts(abs_actions, w2p):
    absflat = np.zeros((B * A, B), np.float32)
    absflat[np.arange(B * A), np.arange(B * A) // A] = abs_actions.reshape(-1)
    _CACHE["absflat"] = np.ascontiguousarray(absflat.reshape(4, P, B)).astype(BF16)
    _CACHE["iota16"] = (np.arange(P, dtype=np.float32) % A).reshape(P, 1)
    _CACHE["w2p"] = np.ascontiguousarray(w2p).astype(BF16)


# revision 37
# speedup vs baseline: 1.3532x; 1.3434x over previous
"""Trainium2 Bass kernel for the Decoder (gather + shared-MLP over agents).

Math:
  assigned[b,n] = abs_actions[b, assign[b,n]]                    (gather, A=16)
  out[b,n,:]    = relu(assigned[b,n]*W1[0,:] + emb[n,:]@W1[1:,:] + b1) @ W2 + b2

Factorization (N sharded 8 ways -> NC=1250 per core, h on partitions):
  embW[n,h] = emb[n,:]@W1[1:,h] (+ b1 folded into emb on host when nonzero)
  relu(x) @ W2 decomposes via max(a,b) = (a+b)/2 + |a-b|/2 with
  a = s*w0[h], b = -embW[n,h]:
    out[b,n,o] = sum_h W2[h,o]*max(s*w0[h], -embW[n,h]) + corr[n,o]
               = DEVICE: sum_h (|w0[h]|W2[h,o]/2) * |s[b,n] - qs[n,h]|
               + HOST:   s[b,n]*g[o]/2 + c[n,o] + corr[n,o]
    qs = -embW/w0,  g = W2.T@w0,  c = -(embW@W2)/2,  corr = embW@W2 + b2.
  The |.| form removes the per-partition sign entirely: every partition
  consumes the SAME broadcast s, so no per-partition scaling op is needed.
  h's with |w0| ~ 0 are excluded from the device (exact on host).

Device pipeline per batch b:
  DMA    : s[b,:] -> partition-0 staging (tiny DMA)
  bcast  : GPSIMD partition_broadcast (even b) OR PE rank-1 matmul with a
           ones vector -> PSUM -> ACT cast (odd b) - splits the broadcast
           load across engines
  DVE    : d = sbc - qs (one fused tensor_tensor over both K-halves, 2x
           bf16), then |d| in place (tensor_scalar abs_max 0, 4x bf16)
  PE     : pso[32j+o, nn] = sum_h |d|[h, 320j+nn]*W2'[h,o] (8 bf16 matmuls,
           col-packed 4x via tile_position)
  ACT    : drain PSUM -> SBUF bf16; one [128,320] DMA out per batch

The gather is a one-hot matmul: onehot[(b,a), n] = (assign[b,n]==a) built by
one DVE compare per 128-row tile, contracted with a block-diagonal
abs_actions matrix on the PE.
"""

import sys

sys.path.insert(0, "/opt/trn_rl_repo")

import numpy as np
import ml_dtypes

import concourse.bass as bass
import concourse.tile as tile
import concourse.mybir as mybir
from concourse import bacc
from concourse.bass_utils import run_bass_kernel_spmd

BF16 = ml_dtypes.bfloat16

B, A, N, E, H, OUT = 32, 16, 10000, 256, 256, 2
NCORES = 8
NC = N // NCORES  # 1250 real columns per core
NP = 1280  # padded to 4 * 320 for regular chunking
P = 128

CH = [0, 512, 1024, NP]  # chunks for matmuls staged through PSUM
CG = [0, 320, 640, 960, NP]  # column groups for the col-packed consume

_CACHE = {}


def build_program():
    """Build the Bass/Tile program once (shared by all 8 cores, SPMD)."""
    nc = bacc.Bacc("TRN2", target_bir_lowering=False, debug=False)
    f32 = mybir.dt.float32
    bf16 = mybir.dt.bfloat16

    d_q = nc.dram_tensor("qtab", (2, P, NP), bf16, kind="ExternalInput").ap()
    d_arep = nc.dram_tensor("assign_rep", (4, P, NP), bf16, kind="ExternalInput").ap()
    d_absf = nc.dram_tensor("absflat", (4, P, B), bf16, kind="ExternalInput").ap()
    d_iota = nc.dram_tensor("iota16", (P, 1), f32, kind="ExternalInput").ap()
    d_ones = nc.dram_tensor("ones1", (1, P), bf16, kind="ExternalInput").ap()
    d_w2p = nc.dram_tensor("w2p", (2, P, 32), bf16, kind="ExternalInput").ap()
    d_out = nc.dram_tensor("out", (B, P, 320), bf16, kind="ExternalOutput").ap()

    mm = mybir.AluOpType

    with tile.TileContext(nc) as tc:
        with (
            tc.tile_pool(name="const", bufs=1) as cpool,
            tc.tile_pool(name="work", bufs=1) as wpool,
            tc.tile_pool(name="sfl", bufs=4) as sflp,
            tc.tile_pool(name="sbc", bufs=4) as sbcp,
            tc.tile_pool(name="tt", bufs=4) as ttp,
            tc.tile_pool(name="ostg", bufs=4) as ostgp,
            tc.tile_pool(name="ps_pro", bufs=3, space="PSUM") as pspro,
            tc.tile_pool(name="ps_out", bufs=4, space="PSUM") as psout,
        ):
            # ---- load constants / inputs ----
            qtab = cpool.tile([P, 2, NP], bf16)
            arep = cpool.tile([P, 4, NP], bf16)
            absf = cpool.tile([P, 4, B], bf16)
            iota = cpool.tile([P, 1], f32)
            ones1 = cpool.tile([1, P], bf16)
            w2p = cpool.tile([P, 2, 32], bf16)

            for k in range(2):
                nc.sync.dma_start(qtab[:, k, :], d_q[k])
                nc.sync.dma_start(w2p[:, k, :], d_w2p[k])
            for t in range(4):
                nc.sync.dma_start(arep[:, t, :], d_arep[t])
                nc.scalar.dma_start(absf[:, t, :], d_absf[t])
            nc.scalar.dma_start(iota[:], d_iota[:])
            nc.scalar.dma_start(ones1[:], d_ones[:])

            # ---- working tensors ----
            onehot = wpool.tile([P, 4, NP], bf16)
            s_all = wpool.tile([B, NP], bf16)

            # ---- one-hot of assignments: onehot[(b,a), n] = (assign[b,n]==a)
            for t in range(4):
                nc.vector.tensor_scalar(
                    onehot[:, t, :], arep[:, t, :], iota[:, 0:1], None, mm.is_equal
                )

            # ---- gather s[b,n] = abs_actions[b, assign[b,n]] via matmul ----
            for ci in range(3):
                w = CH[ci + 1] - CH[ci]
                ps = pspro.tile([P, 512], f32, tag="ps_s")
                for t in range(4):
                    nc.tensor.matmul(
                        ps[:B, :w],
                        absf[:, t, :],
                        onehot[:, t, CH[ci] : CH[ci + 1]],
                        start=(t == 0),
                        stop=(t == 3),
                    )
                nc.scalar.copy(s_all[:, CH[ci] : CH[ci + 1]], ps[:B, :w])

            # ---- main loop over batches ----
            for b in range(B):
                # stage s[b,:] at partition 0 (tiny DMA)
                sfl = sflp.tile([1, NP], bf16, tag="sfl")
                nc.scalar.dma_start(sfl[:], s_all[b : b + 1, :])

                # broadcast to 128 partitions: alternate GPSIMD and
                # PE-rank-1-matmul+ACT-cast to split the load
                sbc = sbcp.tile([P, NP], bf16, tag="sbc")
                if b % 2 == 0:
                    nc.gpsimd.partition_broadcast(sbc[:], sfl[0:1, :])
                else:
                    for ci in range(3):
                        w = CH[ci + 1] - CH[ci]
                        ps = pspro.tile([P, 512], f32, tag="ps_s")
                        nc.tensor.matmul(
                            ps[:, :w],
                            ones1[0:1, :],
                            sfl[0:1, CH[ci] : CH[ci + 1]],
                            start=True,
                            stop=True,
                        )
                        nc.scalar.copy(sbc[:, CH[ci] : CH[ci + 1]], ps[:, :w])

                # d = s - qs over both K-halves, then |d| in place
                tt = ttp.tile([P, 2, NP], bf16, tag="tt")
                nc.vector.tensor_tensor(
                    tt[:],
                    sbc[:].unsqueeze(1).broadcast_to([P, 2, NP]),
                    qtab[:],
                    mm.subtract,
                )
                # |d| in place: clear the bf16 sign bit (uint16 AND 0x7fff)
                ttu = tt[:].bitcast(mybir.dt.uint16)
                nc.vector.tensor_scalar(ttu, ttu, 0x7FFF, None, mm.bitwise_and)

                pso = psout.tile([P, 320], f32, tag="pso")
                for j in range(4):
                    for k in range(2):
                        nc.tensor.matmul(
                            pso[32 * j : 32 * j + 32, :],
                            w2p[:, k, :],
                            tt[:, k, CG[j] : CG[j + 1]],
                            start=(k == 0),
                            stop=(k == 1),
                            tile_position=(0, 32 * j),
                        )

                ostg = ostgp.tile([P, 320], bf16, tag="ostg")
                nc.scalar.copy(ostg[:], pso[:])
                nc.sync.dma_start(d_out[b], ostg[:])

    nc.compile()
    return nc


def prep_inputs(abs_actions, assignments, q_host):
    """Per-core input dicts. q_host: (2, 128, N) bf16 qs table."""
    in_maps = []
    for c in range(NCORES):
        sl = slice(c * NC, (c + 1) * NC)
        a_sl = np.zeros((B, NP), np.int32)
        a_sl[:, :NC] = assignments[:, sl]
        arep = np.ascontiguousarray(
            a_sl[np.arange(B * A) // A].reshape(4, P, NP)
        ).astype(BF16)
        qc = np.zeros((2, P, NP), BF16)
        qc[:, :, :NC] = q_host[:, :, sl]
        in_maps.append(
            {
                "qtab": qc,
                "assign_rep": arep,
                "absflat": _CACHE["absflat"],
                "iota16": _CACHE["iota16"],
                "ones1": _CACHE["ones1"],
                "w2p": _CACHE["w2p"],
            }
        )
    return in_maps


def kernel(abs_actions, abstract_agent_assignments, emb, W1, b1, W2, b2):
    abs_actions = np.asarray(abs_actions, np.float32)
    assign = np.asarray(abstract_agent_assignments).astype(np.int32)
    emb = np.asarray(emb, np.float32)
    W1 = np.asarray(W1, np.float32)
    b1 = np.asarray(b1, np.float32)
    W2 = np.asarray(W2, np.float32)
    b2 = np.asarray(b2, np.float32)

    # Fold b1 into emb: (emb + 1 v^T) @ W1[1:] = emb@W1[1:] + 1 b1^T when
    # W1[1:].T v = b1.  Exact for full-rank square W1[1:]; b1 == 0 here anyway.
    if np.any(b1 != 0):
        v = np.linalg.lstsq(W1[1:].T, b1, rcond=None)[0]
        if not np.allclose(W1[1:].T @ v, b1, atol=1e-5):
            raise ValueError("cannot fold nonzero b1 exactly")
        emb = emb + v[None, :]

    # Weight-only precomputes.
    w0 = W1[0, :]  # (H,)
    embW = emb @ W1[1:]  # (N, H) f32
    corr = embW @ W2 + b2  # (N, OUT) f32

    # Device handles h's with usable |w0| via the abs decomposition; the
    # rest (|w0| ~ 0, division unstable) are exact on the host.
    absw0 = np.abs(w0)
    tiny = absw0 < 1e-5 * max(absw0.max(), 1e-30)
    dev_hs = np.nonzero(~tiny)[0]
    host_hs = list(np.nonzero(tiny)[0])
    nd = len(dev_hs)
    assert nd <= 2 * P

    # slot tables: slot (k, p) <- dev_hs[k*P + p]
    q_host = np.zeros((2, P, N), np.float32)
    w2p = np.zeros((2, P, 32), np.float32)
    for k in range(2):
        hs = dev_hs[k * P : min((k + 1) * P, nd)]
        m = len(hs)
        q_host[k, :m] = (-embW[:, hs] / w0[hs][None, :]).T
        w2p[k, :m, :OUT] = 0.5 * absw0[hs][:, None] * W2[hs, :]
    q_host = q_host.astype(BF16)

    # host-side linear terms
    g = W2[dev_hs].T @ w0[dev_hs]  # (OUT,)
    host_nd = corr - 0.5 * (embW[:, dev_hs] @ W2[dev_hs])  # (N, OUT)

    _build_consts(abs_actions, w2p)

    if "nc" not in _CACHE:
        _CACHE["nc"] = build_program()
    nc = _CACHE["nc"]

    in_maps = prep_inputs(abs_actions, assign, q_host)
    _CACHE["in_maps"] = in_maps
    res = run_bass_kernel_spmd(nc, in_maps, list(range(NCORES))).results
    outs = np.stack(
        [np.asarray(res[c]["out"]).astype(np.float32) for c in range(NCORES)]
    )
    # outs: (8, B, 128, 320); row 32j+o, col nn -> out[b, c*1250 + 320j + nn, o]
    outs = outs.reshape(NCORES, B, 4, 32, 320)[:, :, :, :OUT, :]  # (8,B,4,2,320)
    outs = outs.transpose(1, 0, 2, 4, 3).reshape(B, NCORES, NP, OUT)
    out = np.ascontiguousarray(outs[:, :, :NC, :].reshape(B, N, OUT))

    # host-side: linear rank-1 term, N-term, and any host-exact h's
    s = np.take_along_axis(abs_actions, assign, axis=1)  # (B, N)
    out += host_nd[None]
    out += 0.5 * s[:, :, None] * g[None, None, :]
    for h in host_hs:
        relu_h = np.maximum(s * w0[h], -embW[:, h][None, :])  # (B, N)
        out += relu_h[:, :, None] * W2[h][None, None, :]
    return out


def _build_consts(abs_actions, w2p):
    absflat = np.zeros((B * A, B), np.float32)
    absflat[np.arange(B * A), np.arange(B * A) // A] = abs_actions.reshape(-1)
    _CACHE["absflat"] = np.ascontiguousarray(absflat.reshape(4, P, B)).astype(BF16)
    _CACHE["iota16"] = (np.arange(P, dtype=np.float32) % A).reshape(P, 1)
    _CACHE["ones1"] = np.ones((1, P), BF16)
    _CACHE["w2p"] = np.ascontiguousarray(w2p).astype(BF16)


# revision 40
# speedup vs baseline: 1.3758x; 1.0167x over previous
"""Trainium2 Bass kernel for the Decoder (gather + shared-MLP over agents).

Math:
  assigned[b,n] = abs_actions[b, assign[b,n]]                    (gather, A=16)
  out[b,n,:]    = relu(assigned[b,n]*W1[0,:] + emb[n,:]@W1[1:,:] + b1) @ W2 + b2

Factorization (N sharded 8 ways -> NC=1250 per core, h on partitions):
  embW[n,h] = emb[n,:]@W1[1:,h] (+ b1 folded into emb on host when nonzero)
  relu(x) @ W2 decomposes via max(a,b) = (a+b)/2 + |a-b|/2 with
  a = s*w0[h], b = -embW[n,h]:
    out[b,n,o] = sum_h W2[h,o]*max(s*w0[h], -embW[n,h]) + corr[n,o]
               = DEVICE: sum_h (|w0[h]|W2[h,o]/2) * |s[b,n] - qs[n,h]|
               + HOST:   s[b,n]*g[o]/2 + c[n,o] + corr[n,o]
    qs = -embW/w0,  g = W2.T@w0,  c = -(embW@W2)/2,  corr = embW@W2 + b2.
  The |.| form removes the per-partition sign entirely: every partition
  consumes the SAME broadcast s, so no per-partition scaling op is needed.
  h's with |w0| ~ 0 are excluded from the device (exact on host).

Device pipeline per batch b:
  DMA    : s[b,:] -> partition-0 staging (tiny DMA)
  bcast  : GPSIMD partition_broadcast (even b) OR PE rank-1 matmul with a
           ones vector -> PSUM -> ACT cast (odd b) - splits the broadcast
           load across engines
  DVE    : d = sbc - qs (one fused tensor_tensor over both K-halves, 2x
           bf16), then |d| in place (tensor_scalar abs_max 0, 4x bf16)
  PE     : pso[32j+o, nn] = sum_h |d|[h, 320j+nn]*W2'[h,o] (8 bf16 matmuls,
           col-packed 4x via tile_position)
  ACT    : drain PSUM -> SBUF bf16; one [128,320] DMA out per batch

The gather is a one-hot matmul: onehot[(b,a), n] = (assign[b,n]==a) built by
one DVE compare per 128-row tile, contracted with a block-diagonal
abs_actions matrix on the PE.
"""

import sys

sys.path.insert(0, "/opt/trn_rl_repo")

import numpy as np
import ml_dtypes

import concourse.bass as bass
import concourse.tile as tile
import concourse.mybir as mybir
from concourse import bacc
from concourse.bass_utils import run_bass_kernel_spmd

BF16 = ml_dtypes.bfloat16

B, A, N, E, H, OUT = 32, 16, 10000, 256, 256, 2
NCORES = 8
NC = N // NCORES  # 1250 real columns per core
NP = 1280  # padded to 4 * 320 for regular chunking
P = 128

CH = [0, 512, 1024, NP]  # chunks for matmuls staged through PSUM
CG = [0, 320, 640, 960, NP]  # column groups for the col-packed consume

_CACHE = {}


def build_program():
    """Build the Bass/Tile program once (shared by all 8 cores, SPMD)."""
    nc = bacc.Bacc("TRN2", target_bir_lowering=False, debug=False)
    f32 = mybir.dt.float32
    bf16 = mybir.dt.bfloat16

    d_q = nc.dram_tensor("qtab", (2, P, NP), bf16, kind="ExternalInput").ap()
    d_arep = nc.dram_tensor("assign_rep", (4, P, NP), bf16, kind="ExternalInput").ap()
    d_absf = nc.dram_tensor("absflat", (4, P, B), bf16, kind="ExternalInput").ap()
    d_iota = nc.dram_tensor("iota16", (P, 1), f32, kind="ExternalInput").ap()
    d_ones = nc.dram_tensor("ones1", (1, P), bf16, kind="ExternalInput").ap()
    d_w2p = nc.dram_tensor("w2p", (2, P, 32), bf16, kind="ExternalInput").ap()
    d_out = nc.dram_tensor("out", (B, P, 320), bf16, kind="ExternalOutput").ap()

    mm = mybir.AluOpType

    with tile.TileContext(nc) as tc:
        with (
            tc.tile_pool(name="const", bufs=1) as cpool,
            tc.tile_pool(name="work", bufs=1) as wpool,
            tc.tile_pool(name="sfl", bufs=4) as sflp,
            tc.tile_pool(name="sbc", bufs=4) as sbcp,
            tc.tile_pool(name="td", bufs=4) as tdp,
            tc.tile_pool(name="tt", bufs=4) as ttp,
            tc.tile_pool(name="ostg", bufs=4) as ostgp,
            tc.tile_pool(name="ps_pro", bufs=3, space="PSUM") as pspro,
            tc.tile_pool(name="ps_out", bufs=4, space="PSUM") as psout,
        ):
            # ---- load constants / inputs ----
            qtab = cpool.tile([P, 2, NP], bf16)
            arep = cpool.tile([P, 4, NP], bf16)
            absf = cpool.tile([P, 4, B], bf16)
            iota = cpool.tile([P, 1], f32)
            ones1 = cpool.tile([1, P], bf16)
            w2p = cpool.tile([P, 2, 32], bf16)

            for k in range(2):
                nc.sync.dma_start(qtab[:, k, :], d_q[k])
                nc.sync.dma_start(w2p[:, k, :], d_w2p[k])
            for t in range(4):
                nc.sync.dma_start(arep[:, t, :], d_arep[t])
                nc.scalar.dma_start(absf[:, t, :], d_absf[t])
            nc.scalar.dma_start(iota[:], d_iota[:])
            nc.scalar.dma_start(ones1[:], d_ones[:])

            # ---- working tensors ----
            onehot = wpool.tile([P, 4, NP], bf16)
            s_all = wpool.tile([B, NP], bf16)

            # ---- one-hot of assignments: onehot[(b,a), n] = (assign[b,n]==a)
            for t in range(4):
                nc.vector.tensor_scalar(
                    onehot[:, t, :], arep[:, t, :], iota[:, 0:1], None, mm.is_equal
                )

            # ---- gather s[b,n] = abs_actions[b, assign[b,n]] via matmul ----
            for ci in range(3):
                w = CH[ci + 1] - CH[ci]
                ps = pspro.tile([P, 512], f32, tag="ps_s")
                for t in range(4):
                    nc.tensor.matmul(
                        ps[:B, :w],
                        absf[:, t, :],
                        onehot[:, t, CH[ci] : CH[ci + 1]],
                        start=(t == 0),
                        stop=(t == 3),
                    )
                nc.scalar.copy(s_all[:, CH[ci] : CH[ci + 1]], ps[:B, :w])

            # ---- main loop over batches ----
            for b in range(B):
                # stage s[b,:] at partition 0 (tiny DMA)
                sfl = sflp.tile([1, NP], bf16, tag="sfl")
                nc.scalar.dma_start(sfl[:], s_all[b : b + 1, :])

                # broadcast to 128 partitions: alternate GPSIMD and
                # PE-rank-1-matmul+ACT-cast to split the load
                sbc = sbcp.tile([P, NP], bf16, tag="sbc")
                if b % 4 == 0:
                    nc.gpsimd.partition_broadcast(sbc[:], sfl[0:1, :])
                else:
                    for ci in range(3):
                        w = CH[ci + 1] - CH[ci]
                        ps = pspro.tile([P, 512], f32, tag="ps_s")
                        nc.tensor.matmul(
                            ps[:, :w],
                            ones1[0:1, :],
                            sfl[0:1, CH[ci] : CH[ci + 1]],
                            start=True,
                            stop=True,
                        )
                        nc.scalar.copy(sbc[:, CH[ci] : CH[ci + 1]], ps[:, :w])

                # d = s - qs over both K-halves, then |d| into a fresh tile
                # (sign-clear on uint32 pairs: single-src 2x_2p eligible)
                td = tdp.tile([P, 2, NP], bf16, tag="td")
                nc.vector.tensor_tensor(
                    td[:],
                    sbc[:].unsqueeze(1).broadcast_to([P, 2, NP]),
                    qtab[:],
                    mm.subtract,
                )
                tt = ttp.tile([P, 2, NP], bf16, tag="tt")
                nc.vector.tensor_scalar(
                    tt[:].bitcast(mybir.dt.uint32),
                    td[:].bitcast(mybir.dt.uint32),
                    0x7FFF7FFF,
                    None,
                    mm.bitwise_and,
                )

                pso = psout.tile([P, 320], f32, tag="pso")
                for j in range(4):
                    for k in range(2):
                        nc.tensor.matmul(
                            pso[32 * j : 32 * j + 32, :],
                            w2p[:, k, :],
                            tt[:, k, CG[j] : CG[j + 1]],
                            start=(k == 0),
                            stop=(k == 1),
                            tile_position=(0, 32 * j),
                        )

                ostg = ostgp.tile([P, 320], bf16, tag="ostg")
                nc.scalar.copy(ostg[:], pso[:])
                nc.sync.dma_start(d_out[b], ostg[:])

    nc.compile()
    return nc


def prep_inputs(abs_actions, assignments, q_host):
    """Per-core input dicts. q_host: (2, 128, N) bf16 qs table."""
    in_maps = []
    for c in range(NCORES):
        sl = slice(c * NC, (c + 1) * NC)
        a_sl = np.zeros((B, NP), np.int32)
        a_sl[:, :NC] = assignments[:, sl]
        arep = np.ascontiguousarray(
            a_sl[np.arange(B * A) // A].reshape(4, P, NP)
        ).astype(BF16)
        qc = np.zeros((2, P, NP), BF16)
        qc[:, :, :NC] = q_host[:, :, sl]
        in_maps.append(
            {
                "qtab": qc,
                "assign_rep": arep,
                "absflat": _CACHE["absflat"],
                "iota16": _CACHE["iota16"],
                "ones1": _CACHE["ones1"],
                "w2p": _CACHE["w2p"],
            }
        )
    return in_maps


def kernel(abs_actions, abstract_agent_assignments, emb, W1, b1, W2, b2):
    abs_actions = np.asarray(abs_actions, np.float32)
    assign = np.asarray(abstract_agent_assignments).astype(np.int32)
    emb = np.asarray(emb, np.float32)
    W1 = np.asarray(W1, np.float32)
    b1 = np.asarray(b1, np.float32)
    W2 = np.asarray(W2, np.float32)
    b2 = np.asarray(b2, np.float32)

    # Fold b1 into emb: (emb + 1 v^T) @ W1[1:] = emb@W1[1:] + 1 b1^T when
    # W1[1:].T v = b1.  Exact for full-rank square W1[1:]; b1 == 0 here anyway.
    if np.any(b1 != 0):
        v = np.linalg.lstsq(W1[1:].T, b1, rcond=None)[0]
        if not np.allclose(W1[1:].T @ v, b1, atol=1e-5):
            raise ValueError("cannot fold nonzero b1 exactly")
        emb = emb + v[None, :]

    # Weight-only precomputes.
    w0 = W1[0, :]  # (H,)
    embW = emb @ W1[1:]  # (N, H) f32
    corr = embW @ W2 + b2  # (N, OUT) f32

    # Device handles h's with usable |w0| via the abs decomposition; the
    # rest (|w0| ~ 0, division unstable) are exact on the host.
    absw0 = np.abs(w0)
    tiny = absw0 < 1e-5 * max(absw0.max(), 1e-30)
    dev_hs = np.nonzero(~tiny)[0]
    host_hs = list(np.nonzero(tiny)[0])
    nd = len(dev_hs)
    assert nd <= 2 * P

    # slot tables: slot (k, p) <- dev_hs[k*P + p]
    q_host = np.zeros((2, P, N), np.float32)
    w2p = np.zeros((2, P, 32), np.float32)
    for k in range(2):
        hs = dev_hs[k * P : min((k + 1) * P, nd)]
        m = len(hs)
        q_host[k, :m] = (-embW[:, hs] / w0[hs][None, :]).T
        w2p[k, :m, :OUT] = 0.5 * absw0[hs][:, None] * W2[hs, :]
    q_host = q_host.astype(BF16)

    # host-side linear terms
    g = W2[dev_hs].T @ w0[dev_hs]  # (OUT,)
    host_nd = corr - 0.5 * (embW[:, dev_hs] @ W2[dev_hs])  # (N, OUT)

    _build_consts(abs_actions, w2p)

    if "nc" not in _CACHE:
        _CACHE["nc"] = build_program()
    nc = _CACHE["nc"]

    in_maps = prep_inputs(abs_actions, assign, q_host)
    _CACHE["in_maps"] = in_maps
    res = run_bass_kernel_spmd(nc, in_maps, list(range(NCORES))).results
    outs = np.stack(
        [np.asarray(res[c]["out"]).astype(np.float32) for c in range(NCORES)]
    )
    # outs: (8, B, 128, 320); row 32j+o, col nn -> out[b, c*1250 + 320j + nn, o]
    outs = outs.reshape(NCORES, B, 4, 32, 320)[:, :, :, :OUT, :]  # (8,B,4,2,320)
    outs = outs.transpose(1, 0, 2, 4, 3).reshape(B, NCORES, NP, OUT)
    out = np.ascontiguousarray(outs[:, :, :NC, :].reshape(B, N, OUT))

    # host-side: linear rank-1 term, N-term, and any host-exact h's
    s = np.take_along_axis(abs_actions, assign, axis=1)  # (B, N)
    out += host_nd[None]
    out += 0.5 * s[:, :, None] * g[None, None, :]
    for h in host_hs:
        relu_h = np.maximum(s * w0[h], -embW[:, h][None, :])  # (B, N)
        out += relu_h[:, :, None] * W2[h][None, None, :]
    return out


def _build_consts(abs_actions, w2p):
    absflat = np.zeros((B * A, B), np.float32)
    absflat[np.arange(B * A), np.arange(B * A) // A] = abs_actions.reshape(-1)
    _CACHE["absflat"] = np.ascontiguousarray(absflat.reshape(4, P, B)).astype(BF16)
    _CACHE["iota16"] = (np.arange(P, dtype=np.float32) % A).reshape(P, 1)
    _CACHE["ones1"] = np.ones((1, P), BF16)
    _CACHE["w2p"] = np.ascontiguousarray(w2p).astype(BF16)


# revision 43
# speedup vs baseline: 1.4208x; 1.0327x over previous
"""Trainium2 Bass kernel for the Decoder (gather + shared-MLP over agents).

Math:
  assigned[b,n] = abs_actions[b, assign[b,n]]                    (gather, A=16)
  out[b,n,:]    = relu(assigned[b,n]*W1[0,:] + emb[n,:]@W1[1:,:] + b1) @ W2 + b2

Factorization (N sharded 8 ways -> NC=1250 per core, h on partitions):
  embW[n,h] = emb[n,:]@W1[1:,h] (+ b1 folded into emb on host when nonzero)
  relu(x) @ W2 decomposes via max(a,b) = (a+b)/2 + |a-b|/2 with
  a = s*w0[h], b = -embW[n,h]:
    out[b,n,o] = sum_h W2[h,o]*max(s*w0[h], -embW[n,h]) + corr[n,o]
               = DEVICE: sum_h (|w0[h]|W2[h,o]/2) * |s[b,n] - qs[n,h]|
               + HOST:   s[b,n]*g[o]/2 + c[n,o] + corr[n,o]
    qs = -embW/w0,  g = W2.T@w0,  c = -(embW@W2)/2,  corr = embW@W2 + b2.
  The |.| form removes the per-partition sign entirely: every partition
  consumes the SAME broadcast s, so no per-partition scaling op is needed.
  h's with |w0| ~ 0 are excluded from the device (exact on host).

Device pipeline per batch b:
  DMA    : s[b,:] -> partition-0 staging (tiny DMA)
  bcast  : GPSIMD partition_broadcast (even b) OR PE rank-1 matmul with a
           ones vector -> PSUM -> ACT cast (odd b) - splits the broadcast
           load across engines
  DVE    : d = sbc - qs (one fused tensor_tensor over both K-halves, 2x
           bf16), then |d| in place (tensor_scalar abs_max 0, 4x bf16)
  PE     : pso[32j+o, nn] = sum_h |d|[h, 320j+nn]*W2'[h,o] (8 bf16 matmuls,
           col-packed 4x via tile_position)
  ACT    : drain PSUM -> SBUF bf16; one [128,320] DMA out per batch

The gather is a one-hot matmul: onehot[(b,a), n] = (assign[b,n]==a) built by
one DVE compare per 128-row tile, contracted with a block-diagonal
abs_actions matrix on the PE.
"""

import sys

sys.path.insert(0, "/opt/trn_rl_repo")

import numpy as np
import ml_dtypes

import concourse.bass as bass
import concourse.tile as tile
import concourse.mybir as mybir
from concourse import bacc
from concourse.bass_utils import run_bass_kernel_spmd

BF16 = ml_dtypes.bfloat16

B, A, N, E, H, OUT = 32, 16, 10000, 256, 256, 2
NCORES = 8
NC = N // NCORES  # 1250 real columns per core
NP = 1280  # padded to 4 * 320 for regular chunking
P = 128

CH = [0, 512, 1024, NP]  # chunks for matmuls staged through PSUM
CG = [0, 320, 640, 960, NP]  # column groups for the col-packed consume

_CACHE = {}


def build_program():
    """Build the Bass/Tile program once (shared by all 8 cores, SPMD)."""
    nc = bacc.Bacc("TRN2", target_bir_lowering=False, debug=False)
    f32 = mybir.dt.float32
    bf16 = mybir.dt.bfloat16

    d_q = nc.dram_tensor("qtab", (2, P, NP), bf16, kind="ExternalInput").ap()
    d_arep = nc.dram_tensor("assign_rep", (4, P, NP), bf16, kind="ExternalInput").ap()
    d_absf = nc.dram_tensor("absflat", (4, P, B), bf16, kind="ExternalInput").ap()
    d_iota = nc.dram_tensor("iota16", (P, 1), f32, kind="ExternalInput").ap()
    d_ones = nc.dram_tensor("ones1", (1, P), bf16, kind="ExternalInput").ap()
    d_w2p = nc.dram_tensor("w2p", (2, P, 32), bf16, kind="ExternalInput").ap()
    d_out = nc.dram_tensor("out", (B, P, 320), bf16, kind="ExternalOutput").ap()

    mm = mybir.AluOpType

    with tile.TileContext(nc) as tc:
        with (
            tc.tile_pool(name="const", bufs=1) as cpool,
            tc.tile_pool(name="work", bufs=1) as wpool,
            tc.tile_pool(name="sfl", bufs=6) as sflp,
            tc.tile_pool(name="sbc", bufs=6) as sbcp,
            tc.tile_pool(name="td", bufs=4) as tdp,
            tc.tile_pool(name="tt", bufs=4) as ttp,
            tc.tile_pool(name="ostg", bufs=4) as ostgp,
            tc.tile_pool(name="ps_pro", bufs=3, space="PSUM") as pspro,
            tc.tile_pool(name="ps_out", bufs=4, space="PSUM") as psout,
        ):
            # ---- load constants / inputs ----
            qtab = cpool.tile([P, 2, NP], bf16)
            arep = cpool.tile([P, 4, NP], bf16)
            absf = cpool.tile([P, 4, B], bf16)
            iota = cpool.tile([P, 1], f32)
            ones1 = cpool.tile([1, P], bf16)
            w2p = cpool.tile([P, 2, 32], bf16)

            for t in range(4):
                nc.sync.dma_start(arep[:, t, :], d_arep[t])
                nc.scalar.dma_start(absf[:, t, :], d_absf[t])
            nc.scalar.dma_start(iota[:], d_iota[:])
            nc.scalar.dma_start(ones1[:], d_ones[:])
            for k in range(2):
                nc.sync.dma_start(qtab[:, k, :], d_q[k])
                nc.sync.dma_start(w2p[:, k, :], d_w2p[k])

            # ---- working tensors ----
            onehot = wpool.tile([P, 4, NP], bf16)
            s_all = wpool.tile([B, NP], bf16)

            # ---- one-hot of assignments: onehot[(b,a), n] = (assign[b,n]==a)
            for t in range(4):
                nc.vector.tensor_scalar(
                    onehot[:, t, :], arep[:, t, :], iota[:, 0:1], None, mm.is_equal
                )

            # ---- gather s[b,n] = abs_actions[b, assign[b,n]] via matmul ----
            for ci in range(3):
                w = CH[ci + 1] - CH[ci]
                ps = pspro.tile([P, 512], f32, tag="ps_s")
                for t in range(4):
                    nc.tensor.matmul(
                        ps[:B, :w],
                        absf[:, t, :],
                        onehot[:, t, CH[ci] : CH[ci + 1]],
                        start=(t == 0),
                        stop=(t == 3),
                    )
                nc.scalar.copy(s_all[:, CH[ci] : CH[ci + 1]], ps[:B, :w])

            # ---- main loop over batches ----
            for b in range(B):
                # stage s[b,:] at partition 0 (tiny DMA)
                sfl = sflp.tile([1, NP], bf16, tag="sfl")
                nc.scalar.dma_start(sfl[:], s_all[b : b + 1, :])

                # broadcast to 128 partitions: alternate GPSIMD and
                # PE-rank-1-matmul+ACT-cast to split the load
                sbc = sbcp.tile([P, NP], bf16, tag="sbc")
                if b % 4 == 0:
                    nc.gpsimd.partition_broadcast(sbc[:], sfl[0:1, :])
                else:
                    for ci in range(3):
                        w = CH[ci + 1] - CH[ci]
                        ps = pspro.tile([P, 512], f32, tag="ps_s")
                        nc.tensor.matmul(
                            ps[:, :w],
                            ones1[0:1, :],
                            sfl[0:1, CH[ci] : CH[ci + 1]],
                            start=True,
                            stop=True,
                        )
                        nc.scalar.copy(sbc[:, CH[ci] : CH[ci + 1]], ps[:, :w])

                # d = s - qs, one plain TT per K-half (unit-stride operands),
                # then |d| into a fresh tile (sign-clear on uint32 pairs)
                td = tdp.tile([P, 2, NP], bf16, tag="td")
                for k in range(2):
                    nc.vector.tensor_tensor(
                        td[:, k, :], sbc[:], qtab[:, k, :], mm.subtract
                    )
                tt = ttp.tile([P, 2, NP], bf16, tag="tt")
                nc.vector.tensor_scalar(
                    tt[:].bitcast(mybir.dt.uint32),
                    td[:].bitcast(mybir.dt.uint32),
                    0x7FFF7FFF,
                    None,
                    mm.bitwise_and,
                )

                pso = psout.tile([P, 320], f32, tag="pso")
                for j in range(4):
                    for k in range(2):
                        nc.tensor.matmul(
                            pso[32 * j : 32 * j + 32, :],
                            w2p[:, k, :],
                            tt[:, k, CG[j] : CG[j + 1]],
                            start=(k == 0),
                            stop=(k == 1),
                            tile_position=(0, 32 * j),
                        )

                ostg = ostgp.tile([P, 320], bf16, tag="ostg")
                nc.scalar.copy(ostg[:], pso[:])
                nc.sync.dma_start(d_out[b], ostg[:])

    nc.compile()
    return nc


def prep_inputs(abs_actions, assignments, q_host):
    """Per-core input dicts. q_host: (2, 128, N) bf16 qs table."""
    in_maps = []
    for c in range(NCORES):
        sl = slice(c * NC, (c + 1) * NC)
        a_sl = np.zeros((B, NP), np.int32)
        a_sl[:, :NC] = assignments[:, sl]
        arep = np.ascontiguousarray(
            a_sl[np.arange(B * A) // A].reshape(4, P, NP)
        ).astype(BF16)
        qc = np.zeros((2, P, NP), BF16)
        qc[:, :, :NC] = q_host[:, :, sl]
        in_maps.append(
            {
                "qtab": qc,
                "assign_rep": arep,
                "absflat": _CACHE["absflat"],
                "iota16": _CACHE["iota16"],
                "ones1": _CACHE["ones1"],
                "w2p": _CACHE["w2p"],
            }
        )
    return in_maps


def kernel(abs_actions, abstract_agent_assignments, emb, W1, b1, W2, b2):
    abs_actions = np.asarray(abs_actions, np.float32)
    assign = np.asarray(abstract_agent_assignments).astype(np.int32)
    emb = np.asarray(emb, np.float32)
    W1 = np.asarray(W1, np.float32)
    b1 = np.asarray(b1, np.float32)
    W2 = np.asarray(W2, np.float32)
    b2 = np.asarray(b2, np.float32)

    # Fold b1 into emb: (emb + 1 v^T) @ W1[1:] = emb@W1[1:] + 1 b1^T when
    # W1[1:].T v = b1.  Exact for full-rank square W1[1:]; b1 == 0 here anyway.
    if np.any(b1 != 0):
        v = np.linalg.lstsq(W1[1:].T, b1, rcond=None)[0]
        if not np.allclose(W1[1:].T @ v, b1, atol=1e-5):
            raise ValueError("cannot fold nonzero b1 exactly")
        emb = emb + v[None, :]

    # Weight-only precomputes.
    w0 = W1[0, :]  # (H,)
    embW = emb @ W1[1:]  # (N, H) f32
    corr = embW @ W2 + b2  # (N, OUT) f32

    # Device handles h's with usable |w0| via the abs decomposition; the
    # rest (|w0| ~ 0, division unstable) are exact on the host.
    absw0 = np.abs(w0)
    tiny = absw0 < 1e-5 * max(absw0.max(), 1e-30)
    dev_hs = np.nonzero(~tiny)[0]
    host_hs = list(np.nonzero(tiny)[0])
    nd = len(dev_hs)
    assert nd <= 2 * P

    # slot tables: slot (k, p) <- dev_hs[k*P + p]
    q_host = np.zeros((2, P, N), np.float32)
    w2p = np.zeros((2, P, 32), np.float32)
    for k in range(2):
        hs = dev_hs[k * P : min((k + 1) * P, nd)]
        m = len(hs)
        q_host[k, :m] = (-embW[:, hs] / w0[hs][None, :]).T
        w2p[k, :m, :OUT] = 0.5 * absw0[hs][:, None] * W2[hs, :]
    q_host = q_host.astype(BF16)

    # host-side linear terms
    g = W2[dev_hs].T @ w0[dev_hs]  # (OUT,)
    host_nd = corr - 0.5 * (embW[:, dev_hs] @ W2[dev_hs])  # (N, OUT)

    _build_consts(abs_actions, w2p)

    if "nc" not in _CACHE:
        _CACHE["nc"] = build_program()
    nc = _CACHE["nc"]

    in_maps = prep_inputs(abs_actions, assign, q_host)
    _CACHE["in_maps"] = in_maps
    res = run_bass_kernel_spmd(nc, in_maps, list(range(NCORES))).results
    outs = np.stack(
        [np.asarray(res[c]["out"]).astype(np.float32) for c in range(NCORES)]
    )
    # outs: (8, B, 128, 320); row 32j+o, col nn -> out[b, c*1250 + 320j + nn, o]
    outs = outs.reshape(NCORES, B, 4, 32, 320)[:, :, :, :OUT, :]  # (8,B,4,2,320)
    outs = outs.transpose(1, 0, 2, 4, 3).reshape(B, NCORES, NP, OUT)
    out = np.ascontiguousarray(outs[:, :, :NC, :].reshape(B, N, OUT))

    # host-side: linear rank-1 term, N-term, and any host-exact h's
    s = np.take_along_axis(abs_actions, assign, axis=1)  # (B, N)
    out += host_nd[None]
    out += 0.5 * s[:, :, None] * g[None, None, :]
    for h in host_hs:
        relu_h = np.maximum(s * w0[h], -embW[:, h][None, :])  # (B, N)
        out += relu_h[:, :, None] * W2[h][None, None, :]
    return out


def _build_consts(abs_actions, w2p):
    absflat = np.zeros((B * A, B), np.float32)
    absflat[np.arange(B * A), np.arange(B * A) // A] = abs_actions.reshape(-1)
    _CACHE["absflat"] = np.ascontiguousarray(absflat.reshape(4, P, B)).astype(BF16)
    _CACHE["iota16"] = (np.arange(P, dtype=np.float32) % A).reshape(P, 1)
    _CACHE["ones1"] = np.ones((1, P), BF16)
    _CACHE["w2p"] = np.ascontiguousarray(w2p).astype(BF16)
